# revision 1
# baseline (speedup 1.0000x reference)
"""DyConv2d (dynamic convolution with SE attention) on 8 TRN2 NeuronCores.

Reference computation (per image):
    attn = softmax(MLP(global_avg_pool(x)) / T)            # [K=4]
    y    = conv3x3(x, W) + bias                            # W: [K*128, 128, 3, 3]
    out  = sum_k attn[k] * y[k]                            # [128, 64, 64]

Key algebraic rewrite: conv is linear in the weights, so
    out = conv3x3(x, sum_k attn[k] * W_k) + sum_k attn[k] * bias_k
which cuts the conv FLOPs by 4x (one 128->128 conv per image instead of
128->512).

Sharding: data-parallel over batch, 2 images per core. The replicated
weights are laid out host-side in the transposed [k, ci, tap, co] order the
TensorE needs (lhsT), so no on-device transposes are required and the
per-tap-group weight DMAs pipeline with the attention computation.

Per-core pipeline (engine assignment keeps the PE the bottleneck):
  1. x DMA (sync HWDGE, 2 halves) -> DVE re-rounds to float32r (the PE's
     full-rate fp32 mode, ~1.5e-4 rel err) into a flat-padded layout and
     emits the SE global sum via accum_out.
  2. Wt DMA (scalar HWDGE queue) in tap-group-major order so the first
     combine group unblocks after ~1/3 of the weight bytes.
  3. Per-image SE MLP on PE (tiny, exact f32); softmax on ACT/DVE with two
     tiny DRAM bounces for the [4,1]->[1,4] transpose and the 128-partition
     attn broadcast (DRAM APs allow partition-stride-0).
  4. Per-image weight combine over k on DVE in 3 groups of 3 taps (fused
     scalar_tensor_tensor chain, final write rounds to f32r), so the conv
     starts right after group 0.
  5. Conv: flat-padded layout with row pitch 65 -> each row's right pad
     aliases the next row's left pad (zero), so every 3x3 tap is one fully
     contiguous fp32r matmul at flat offset dy*65+dx. Tap-major over groups
     of 2-3 row-blocks (PSUM banks), 9 accumulating matmuls per bank,
     N = 7*65+1 = 456 (fp32r requires even N <= 512).
  6. Eviction adds the attn-combined bias on ACT (Identity + bias AP) and
     DMAs out on alternating HWDGE queues.
"""

import sys

sys.path.insert(0, "/opt/trn_rl_repo")

import numpy as np

from concourse import bacc, mybir
import concourse.tile as tile
from concourse.bass_utils import run_bass_kernel_spmd
from concourse.tile_rust import add_dep_helper

B_TOTAL = 16
N_CORES = 8
B = B_TOTAL // N_CORES  # images per core
CI = 128
CO = 128
K = 4
H = W = 64
TEMP = 30.0
F32 = mybir.dt.float32
F32R = mybir.dt.float32r

# raster order: tap i = (i//3, i%3), matching the weff group layout
TAPS = [(i // 3, i % 3) for i in range(9)]
XPL = 65 * 66 + 4  # padded-x flat length (extra zeros absorb window overrun)
BLOCKS = [(h0, 7) for h0 in range(0, 63, 7)] + [(63, 1)]
BGROUPS = [BLOCKS[0:3], BLOCKS[3:6], BLOCKS[6:8], BLOCKS[8:10]]

_NC_CACHE = {}


def build_nc(reps=1):
    nc = bacc.Bacc("TRN2", target_bir_lowering=False)

    x_d = nc.dram_tensor("x2", [B, CI, H, W], F32, kind="ExternalInput")
    wt_d = nc.dram_tensor("weight_t", [K, CI, 9, CO], F32, kind="ExternalInput")
    bc_d = nc.dram_tensor("bias_cos", [CO, K], F32, kind="ExternalInput")
    w1t_d = nc.dram_tensor("se_w1t", [CI, 33], F32, kind="ExternalInput")
    w2t_d = nc.dram_tensor("se_w2t", [33, K], F32, kind="ExternalInput")
    b2_d = nc.dram_tensor("se_b2", [K], F32, kind="ExternalInput")
    y_d = nc.dram_tensor("y2", [B, CO, H, W], F32, kind="ExternalOutput")

    with tile.TileContext(nc) as tc:
        with (
            tc.tile_pool(name="consts", bufs=1) as consts,
            tc.tile_pool(name="ximg", bufs=2) as ximg,
            tc.tile_pool(name="weff", bufs=6) as weffp,
            tc.tile_pool(name="cmb", bufs=2) as cmbp,
            tc.tile_pool(name="sesb", bufs=2) as sesb,
            tc.tile_pool(name="ev", bufs=4) as evp,
            tc.tile_pool(name="cv", bufs=6, space="PSUM") as cvp,
            tc.tile_pool(name="tp", bufs=2, space="PSUM") as tpp,
        ):
            for _ in range(reps):
                build_body(nc, tc, consts, ximg, weffp, cmbp, sesb, evp, cvp,
                           tpp, x_d, wt_d, bc_d, w1t_d, w2t_d, b2_d, y_d)

    nc.compile()
    return nc


def build_body(nc, tc, consts, ximg, weffp, cmbp, sesb, evp, cvp, tpp,
               x_d, wt_d, bc_d, w1t_d, w2t_d, b2_d, y_d):
    pooled = consts.tile([128, B], F32, tag="pooled")
    pool_parts = consts.tile([128, B, 2], F32, tag="pool_parts")
    lg_dram = nc.dram_tensor("lg_bounce", [B, K], F32)
    attn_dram = nc.dram_tensor("attn_bounce", [B, K], F32)
    x_sb = [None, None]
    x_r = [None, None]

    def load_x(b):
        t = ximg.tile([128, H, W], F32, tag=f"x_sb{b}", name=f"x_sb{b}")
        nc.sync.dma_start(out=t[:, 0:32, :], in_=x_d[b, :, 0:32, :])
        nc.sync.dma_start(out=t[:, 32:64, :], in_=x_d[b, :, 32:64, :])
        x_sb[b] = t

    def round_image(b):
        """f32r-round x into the flat-padded layout; accumulate the SE sums."""
        xr = ximg.tile([128, XPL], F32R, tag=f"x_r{b}", name=f"x_r{b}")
        xr_rows = xr[:, 0:65 * 66].rearrange("p (r c) -> p r c", c=65)
        x_flat = x_sb[b].rearrange("p a b -> p (a b)")
        # zero the pad cells; memset can't produce float32r, so use in*0 ops
        for pad_out, pad_in in [
            (xr[:, 0:66], x_flat[:, 0:66]),            # top pad row
            (xr_rows[:, 2:65, 0], x_flat[:, 0:63]),    # left pads
            (xr[:, 65 * 65:XPL], x_flat[:, 0:69]),     # bottom pad row
        ]:
            nc.vector.tensor_scalar(
                out=pad_out, in0=pad_in, scalar1=0.0, scalar2=None,
                op0=mybir.AluOpType.mult,
            )
        for hh in (0, 1):  # round each 32-row half as its DMA lands
            nc.vector.tensor_scalar(
                out=xr_rows[:, 1 + 32 * hh:1 + 32 * (hh + 1), 1:65],
                in0=x_sb[b][:, 32 * hh:32 * (hh + 1), :],
                scalar1=1.0, scalar2=0.0,
                op0=mybir.AluOpType.mult, op1=mybir.AluOpType.add,
                accum_out=pool_parts[:, b, hh:hh + 1],
            )
        nc.vector.tensor_add(pooled[:, b:b + 1], pool_parts[:, b, 0:1],
                             pool_parts[:, b, 1:2])
        x_r[b] = xr

    # ---- weights (already [k, ci, tap, co] from the host), group-major ----
    wt = [consts.tile([128, 9, CO], F32, tag=f"wt{k}", name=f"wt{k}")
          for k in range(K)]

    def load_w_group(g):
        # one contiguous DMA per k (4.6KB/partition runs, max DMA efficiency)
        if g == 0:
            for k in range(K):
                nc.scalar.dma_start(out=wt[k], in_=wt_d[k])

    # tiny SE params first: a few KB that gate the whole attention chain
    w1t_sb = consts.tile([CI, 33], F32, tag="w1t_sb")
    nc.scalar.dma_start(out=w1t_sb, in_=w1t_d[:, :])
    w2t_sb = consts.tile([33, K], F32, tag="w2t_sb")
    nc.scalar.dma_start(out=w2t_sb, in_=w2t_d[:, :])
    b2_sb = consts.tile([K, 1], F32, tag="b2_sb")
    nc.scalar.dma_start(out=b2_sb, in_=b2_d[:].rearrange("(a b) -> a b", b=1))
    bias_cos = consts.tile([CO, K], F32, tag="bias_cos")
    nc.scalar.dma_start(out=bias_cos, in_=bc_d[:, :])
    load_x(0)
    load_w_group(0)
    round_image(0)

    cb_all = consts.tile([128, B], F32, tag="cb_all")

    def se_attn(b):
        """SE MLP + softmax for one image -> attn_bc [128, K]; cb into cb_all."""
        ps_h = tpp.tile([128, 512], F32, tag="tp", name="ps_h")[0:33, 0:1]
        nc.tensor.matmul(ps_h, w1t_sb, pooled[:, b:b + 1], start=True, stop=True)
        h_sb = sesb.tile([33, 1], F32, tag="h_sb")
        nc.scalar.activation(out=h_sb, in_=ps_h,
                             func=mybir.ActivationFunctionType.Relu,
                             scale=1.0 / (H * W))
        ps_lg = tpp.tile([128, 512], F32, tag="tp", name="ps_lg")[0:K, 0:1]
        nc.tensor.matmul(ps_lg, w2t_sb, h_sb, start=True, stop=True)
        lg_sb = sesb.tile([K, 1], F32, tag="lg_sb")
        nc.scalar.activation(out=lg_sb, in_=ps_lg,
                             func=mybir.ActivationFunctionType.Identity,
                             bias=b2_sb[:, 0:1], scale=1.0)
        # [4,1] -> [1,4] via a tiny DRAM bounce (DRAM APs are layout-free)
        nc.sync.dma_start(out=lg_dram[b], in_=lg_sb)
        lgt = sesb.tile([1, K], F32, tag="lgt")
        nc.sync.dma_start(out=lgt, in_=lg_dram[b].rearrange("(a k) -> a k", a=1))
        e_sb = sesb.tile([1, K], F32, tag="e_sb")
        nc.scalar.activation(out=e_sb, in_=lgt,
                             func=mybir.ActivationFunctionType.Exp,
                             scale=1.0 / TEMP)
        s_sb = sesb.tile([1, 1], F32, tag="s_sb")
        nc.vector.reduce_sum(out=s_sb, in_=e_sb, axis=mybir.AxisListType.X)
        r_sb = sesb.tile([1, 1], F32, tag="r_sb")
        nc.vector.reciprocal(out=r_sb, in_=s_sb)
        attn = sesb.tile([1, K], F32, tag="attn")
        nc.vector.tensor_scalar_mul(attn, e_sb, r_sb[:, 0:1])
        # broadcast to 128 partitions via DRAM bounce (partition stride 0)
        nc.sync.dma_start(out=attn_dram[b], in_=attn)
        attn_bc = sesb.tile([128, K], F32, tag="attn_bc")
        nc.sync.dma_start(out=attn_bc, in_=attn_dram[b].partition_broadcast(128))
        # combined bias cb = sum_k attn[k] * bias[k]
        tmp = sesb.tile([128, K], F32, tag="cbtmp")
        nc.vector.tensor_mul(tmp, bias_cos, attn_bc)
        nc.vector.reduce_sum(out=cb_all[:, b:b + 1], in_=tmp,
                             axis=mybir.AxisListType.X)
        return attn_bc

    def combine_group(attn_bc, g):
        """weff_g [128, 3, CO] (f32r) = sum_k attn[k] * wt[k][:, 3g:3g+3, :]"""
        sl = slice(3 * g, 3 * g + 3)
        t0 = cmbp.tile([128, 3, CO], F32, tag="cmb_t")
        nc.vector.tensor_scalar(
            out=t0, in0=wt[0][:, sl, :], scalar1=attn_bc[:, 0:1],
            scalar2=None, op0=mybir.AluOpType.mult)
        t1 = cmbp.tile([128, 3, CO], F32, tag="cmb_t")
        nc.vector.scalar_tensor_tensor(
            out=t1, in0=wt[1][:, sl, :], scalar=attn_bc[:, 1:2], in1=t0,
            op0=mybir.AluOpType.mult, op1=mybir.AluOpType.add)
        t2 = cmbp.tile([128, 3, CO], F32, tag="cmb_t")
        nc.vector.scalar_tensor_tensor(
            out=t2, in0=wt[2][:, sl, :], scalar=attn_bc[:, 2:3], in1=t1,
            op0=mybir.AluOpType.mult, op1=mybir.AluOpType.add)
        wg = weffp.tile([128, 3, CO], F32R, tag="weff")
        last = nc.vector.scalar_tensor_tensor(
            out=wg, in0=wt[3][:, sl, :], scalar=attn_bc[:, 3:4], in1=t2,
            op0=mybir.AluOpType.mult, op1=mybir.AluOpType.add)
        return wg, last

    def conv_image(b, weff_groups):
        xr = x_r[b]
        for gi, grp in enumerate(BGROUPS):
            pss = [cvp.tile([128, 512], F32, tag="cv", name=f"cv{j}")
                   for j in range(len(grp))]
            grows = sum(nr for _, nr in grp)
            gh0 = grp[0][0]
            out_sb = evp.tile([128, 21, W], F32, tag="ev", name="ev_g")
            for i, (ky, kx) in enumerate(TAPS):
                lhsT = weff_groups[i // 3][:, i % 3, :]
                off = (ky - 1) * 65 + (kx - 1)
                for j, (ps, (h0, nr)) in enumerate(zip(pss, grp)):
                    n = nr * 65 + 1  # +1 keeps N even (fp32r requires it)
                    obase = (h0 + 1) * 65 + 1
                    nc.tensor.matmul(
                        ps[:, 0:n], lhsT, xr[:, obase + off:obase + off + n],
                        start=(i == 0), stop=(i == 8),
                    )
                    if i == 8:
                        # evict right away (bias add) into the group staging
                        # tile so the PSUM bank frees while the PE finishes
                        # the remaining stop-tap matmuls
                        r0 = h0 - gh0
                        ps_rows = ps[:, 0:455].rearrange("p (r c) -> p r c",
                                                         c=65)
                        nc.scalar.activation(
                            out=out_sb[:, r0:r0 + nr, :],
                            in_=ps_rows[:, 0:nr, 0:64],
                            func=mybir.ActivationFunctionType.Identity,
                            bias=cb_all[:, b:b + 1], scale=1.0)
            # one large contiguous DMA per block-group (fewer descriptors)
            dma_eng = nc.sync if gi % 2 == 0 else nc.scalar
            dma_eng.dma_start(out=y_d[b, :, gh0:gh0 + grows, :],
                              in_=out_sb[:, 0:grows, :])

    # image 0: SE -> combine groups -> conv (taps of group g follow combine g)
    attn0 = se_attn(0)
    w0 = [combine_group(attn0, g)[0] for g in range(3)]
    load_x(1)
    conv_image(0, w0)
    # image 1 prep (placed after image-0 combine so it can't delay it on DVE)
    round_image(1)
    attn1 = se_attn(1)
    w1 = [combine_group(attn1, g)[0] for g in range(3)]
    conv_image(1, w1)


def get_nc():
    if "nc" not in _NC_CACHE:
        _NC_CACHE["nc"] = build_nc()
    return _NC_CACHE["nc"]


def shard_inputs(x, weight, bias, se_w1, se_w2, se_b2):
    # host-side layout prep of the replicated (batch-independent) params:
    # weight -> [k, ci, tap, co] (the lhsT layout the TensorE consumes)
    w4 = np.ascontiguousarray(weight, np.float32).reshape(K, CO, CI, 3, 3)
    weight_t = np.ascontiguousarray(w4.transpose(0, 2, 3, 4, 1)
                                    .reshape(K, CI, 9, CO))
    common = dict(
        weight_t=weight_t,
        bias_cos=np.ascontiguousarray(
            np.asarray(bias, np.float32).reshape(K, CO).T),
        se_w1t=np.ascontiguousarray(np.asarray(se_w1, np.float32).T),
        se_w2t=np.ascontiguousarray(np.asarray(se_w2, np.float32).T),
        se_b2=np.ascontiguousarray(se_b2, np.float32),
    )
    return [
        dict(x2=np.ascontiguousarray(x[c * B:(c + 1) * B], np.float32), **common)
        for c in range(N_CORES)
    ]


def kernel(x, weight, bias, se_w1, se_w2, se_b2):
    nc = get_nc()
    in_maps = shard_inputs(x, weight, bias, se_w1, se_w2, se_b2)
    res = run_bass_kernel_spmd(nc, in_maps, core_ids=list(range(N_CORES)))
    return np.concatenate([r["y2"] for r in res.results], axis=0)



# revision 50
# speedup vs baseline: 1.6942x; 1.6942x over previous
"""DyConv2d (dynamic convolution with SE attention) on 8 TRN2 NeuronCores.

Reference computation (per image):
    attn = softmax(MLP(global_avg_pool(x)) / T)            # [K=4]
    y    = conv3x3(x, W) + bias                            # W: [K*128, 128, 3, 3]
    out  = sum_k attn[k] * y[k]                            # [128, 64, 64]

Conv is linear in the weights, so out = conv3x3(x, sum_k attn[k] W_k) +
sum_k attn[k] bias_k: one 128->128 conv per image instead of 128->512 (4x
fewer FLOPs). Data-parallel over batch, 2 images per core.

Layout/precision strategy (tolerance is 2e-2; fp16 keeps us ~1e-3):
  * x is zero-padded HOST-side into the flat pitch-65 layout
    [66 rows x 65 cols + 4] as fp16: every DMA lands conv-ready (each row's
    right pad aliases the next row's left pad), no on-chip re-layout, and
    fp16 halves the DMA bytes. fp16 matmul runs at 1 column/cycle, same as
    fp32r, with ~1e-4 input rounding error.
  * weights are host-prepped to the lhsT layout [ky-group][k, ci, kx, co]
    fp16; the per-image combine (4 fused DVE ops per ky-group) emits weff
    fp16 so the conv can start after group 0.
  * output is written fp16 and upcast to f32 on the host (~5e-4 of scale).

Schedule (single core, per-engine):
  * PE: warm-up matmuls on zeros from t~0.5us keep the PE busy so the
    p-state ramp (0.65 -> 2.4 GHz over 3us) completes before the first conv
    matmul; the SE matmuls slot between warm-up batches. Conv: 9 taps x 8
    row-blocks of [8 rows x 64 cols] = 512-column PSUM banks, tap-major so
    each bank accumulates all 9 taps; 7 "A" banks + 1 shared "B" bank
    (which also serves the tiny SE matmuls between images).
  * SE without DRAM bounces: logits come out as a [1,K] row by swapping
    matmul operands (lhsT=h, rhs=w2t); softmax uses host-precomputed
    exp(b2/T); attn is broadcast to 128 partitions with a ones-lhsT matmul.
  * DVE: per-half pooled reduction as x lands, weight combine, half the
    PSUM evictions. ACT: relu/exp, the other half of evictions (fused
    identity+bias+fp16-convert).
  * DMA: inputs on the sync queue (x img0 halves, weight group 0, x img1,
    groups 1-2), consts as one packed blob on the scalar queue; per-block
    output DMAs alternate sync/scalar.
"""

import sys

sys.path.insert(0, "/opt/trn_rl_repo")

import numpy as np

from concourse import bacc, mybir
import concourse.tile as tile
from concourse.bass_utils import run_bass_kernel_spmd

B_TOTAL = 16
N_CORES = 8
B = B_TOTAL // N_CORES  # images per core
CI = 128
CO = 128
K = 4
H = W = 64
HID = 33
TEMP = 30.0
F32 = mybir.dt.float32
F16 = mybir.dt.float16

PITCH = 65
XPL = PITCH * 66 + 4  # padded-x flat length (extra zeros absorb overrun)
NBLK = 8              # row blocks of 8 rows -> N=512 = one PSUM bank
BROWS = 8
NCOL = BROWS * PITCH  # 520 flat elements spanned by one block window

# const blob layout (f32, [128, BLOB_W]): w1t | w2aug | bias_cos
BLOB_W1T = 0                 # [128, 33]
BLOB_W2T = 33                # [34, 4] in partitions 0:34 (row 33 = se_b2)
BLOB_BCOS = 37               # [128, 4]
BLOB_W = 41

_NC_CACHE = {}


def build_nc():
    nc = bacc.Bacc("TRN2", target_bir_lowering=False)

    x_d = nc.dram_tensor("xp", [B, CI, XPL], F16, kind="ExternalInput")
    # weights grouped by ky: [ky][ci, k, kx, co] fp16 (partition-major,
    # matching the SBUF tile layout)
    wg_d = [nc.dram_tensor(f"wg{g}", [CI, K, 3, CO], F16, kind="ExternalInput")
            for g in range(3)]
    blob_d = nc.dram_tensor("cblob", [CI, BLOB_W], F32, kind="ExternalInput")
    y_d = nc.dram_tensor("y2", [B, CO, H, W], F16, kind="ExternalOutput")

    with tile.TileContext(nc) as tc:
        with (
            tc.tile_pool(name="consts", bufs=1) as consts,
            tc.tile_pool(name="ximg", bufs=2) as ximg,
            tc.tile_pool(name="weff", bufs=2) as weffp,
            tc.tile_pool(name="cmb", bufs=2) as cmbp,
            tc.tile_pool(name="sesb", bufs=2) as sesb,
            tc.tile_pool(name="ev", bufs=6) as evp,
            tc.tile_pool(name="cv", bufs=7, space="PSUM") as cvp,
            tc.tile_pool(name="tp", bufs=1, space="PSUM") as tpp,
        ):
            build_body(nc, tc, consts, ximg, weffp, cmbp, sesb, evp, cvp,
                       tpp, x_d, wg_d, blob_d, y_d)

    nc.compile()
    return nc


def build_body(nc, tc, consts, ximg, weffp, cmbp, sesb, evp, cvp, tpp,
               x_d, wg_d, blob_d, y_d):
    # ---- input DMAs (sync queue; x image 0 first, then wg0, x1, wg1-2) ----
    xr = [ximg.tile([CI, XPL], F16, tag=f"xr{b}", name=f"xr{b}")
          for b in range(B)]
    # image 0 in 3 pieces (small last piece => pooled available sooner);
    # chunked pooled reductions below are aligned to these piece bounds
    X0CUTS = [0, 1074, 2147, 3500, XPL]   # reduce-chunk bounds
    X1CUTS = [0, 1074, 2147, 3221, XPL]
    for lo, hi in [(0, 2147), (2147, 3500), (3500, XPL)]:
        nc.sync.dma_start(out=xr[0][:, lo:hi], in_=x_d[0, :, lo:hi])
    wg_sb = [consts.tile([CI, K, 3, CO], F16, tag=f"wg{g}", name=f"wg{g}")
             for g in range(3)]
    nc.sync.dma_start(out=wg_sb[0], in_=wg_d[0][:, :, :, :])
    nc.sync.dma_start(out=wg_sb[1], in_=wg_d[1][:, :, :, :])
    # x image 1 after wg1: its reduce chunks become DVE-ready only after
    # image 0's critical combine chain has started
    for lo, hi in [(0, 2147), (2147, XPL)]:
        nc.sync.dma_start(out=xr[1][:, lo:hi], in_=x_d[1, :, lo:hi])
    nc.sync.dma_start(out=wg_sb[2], in_=wg_d[2][:, :, :, :])

    blob = consts.tile([CI, BLOB_W], F32, tag="blob")
    nc.scalar.dma_start(out=blob, in_=blob_d[:, :])
    w1t_sb = blob[:, BLOB_W1T:BLOB_W1T + HID]
    w2aug_sb = blob[0:HID + 1, BLOB_W2T:BLOB_W2T + K]
    bcos_sb = blob[:, BLOB_BCOS:BLOB_BCOS + K]

    # ---- PE warm-up: zero matmuls keep the p-state ramp going ----
    zl = consts.tile([CI, CO], F16, tag="zlhs")
    nc.gpsimd.memset(zl, 0.0)
    zr = consts.tile([CI, 512], F16, tag="zrhs")
    nc.gpsimd.memset(zr, 0.0)

    def dummies(n, pool=None):
        # mid-chain warm-up batches allocate from the cv pool: the tp bank
        # is held by SE PSUM tiles whose readers would stall a fresh alloc
        for _ in range(n):
            ps = (pool or tpp).tile([128, 512], F32, tag="cv" if pool
                                    else "tp", name="warm")
            nc.tensor.matmul(ps, zl, zr, start=True, stop=True)

    # ---- per-image pooled sums: chunked copy+accum on DVE (immediate
    # scalars get the fast DVE modes; small chunks can't block the
    # scheduler's critical ops) ----
    pooled = consts.tile([CI, B], F32, tag="pooled")
    NRED = 4
    pparts = consts.tile([CI, B, NRED], F32, tag="pparts")
    rscr = [consts.tile([CI, 1400], F16, tag=f"rscr{b}", name=f"rscr{b}")
            for b in range(B)]

    def reduce_image(b):
        cuts = X0CUTS if b == 0 else X1CUTS
        for i in range(NRED):
            o, n = cuts[i], cuts[i + 1] - cuts[i]
            nc.vector.tensor_scalar(
                out=rscr[b][:, 0:n], in0=xr[b][:, o:o + n],
                scalar1=1.0, scalar2=0.0,
                op0=mybir.AluOpType.mult, op1=mybir.AluOpType.add,
                accum_out=pparts[:, b, i:i + 1])
        nc.vector.reduce_sum(out=pooled[:, b:b + 1], in_=pparts[:, b, :],
                             axis=mybir.AxisListType.X)

    e_all = consts.tile([CI, K, B], F32, tag="e_all")
    r_all = consts.tile([CI, B], F32, tag="r_all")
    cb_all = consts.tile([CI, B], F32, tag="cb_all")

    def se_attn(b):
        """SE MLP -> raw softmax exponentials e_all[:, :, b] (broadcast on
        all partitions) and r_all[:, b] = 1/sum(e).

        h is replicated across 128 columns with a stride-0 read in the relu,
        so mm_lg (lhsT = [h; 1] augmented with a ones row that folds se_b2
        into the logits) directly yields logits broadcast over partitions.
        The weight combine consumes e directly (no normalize on the critical
        path); the 1/sum(e) factor is applied by each eviction's scale.
        """
        ps_h = tpp.tile([128, 512], F32, tag="tp", name=f"ps_h{b}")[0:HID, 0:1]
        nc.tensor.matmul(ps_h, w1t_sb, pooled[:, b:b + 1], start=True,
                         stop=True)
        h_aug = sesb.tile([HID + 1, CO], F32, tag="h_aug")
        nc.vector.memset(h_aug, 1.0)  # row HID stays 1.0 (folds se_b2 in)
        nc.scalar.activation(out=h_aug[0:HID, :],
                             in_=ps_h.broadcast_to([HID, CO]),
                             func=mybir.ActivationFunctionType.Relu,
                             scale=1.0 / (H * W))
        ps_lg = tpp.tile([128, 512], F32, tag="tp", name=f"ps_lg{b}")[:, 0:K]
        nc.tensor.matmul(ps_lg, h_aug, w2aug_sb, start=True, stop=True)
        nc.scalar.activation(out=e_all[:, :, b], in_=ps_lg,
                             func=mybir.ActivationFunctionType.Exp,
                             scale=1.0 / TEMP)
        s_sb = sesb.tile([CI, 1], F32, tag="s_sb")
        nc.vector.reduce_sum(out=s_sb, in_=e_all[:, :, b],
                             axis=mybir.AxisListType.X)
        nc.vector.reciprocal(out=r_all[:, b:b + 1], in_=s_sb)

    def emit_cb(b):
        # combined bias cb = r * sum_k e[k]*bias[k*CO+co] (emitted after the
        # critical combine chains; needed only at eviction time)
        tmp = sesb.tile([CI, K], F32, tag="cbtmp", name=f"cbt{b}")
        nc.vector.tensor_mul(tmp, bcos_sb, e_all[:, :, b])
        nc.vector.tensor_reduce(out=cb_all[:, b:b + 1], in_=tmp,
                                axis=mybir.AxisListType.X,
                                op=mybir.AluOpType.add)
        nc.vector.tensor_scalar_mul(cb_all[:, b:b + 1], cb_all[:, b:b + 1],
                                    r_all[:, b:b + 1])

    def combine(b, g, taps):
        """weff[b][:, taps, :] = sum_k e[k] * wg_sb[g][:, k, taps%3, :]"""
        a = e_all[:, :, b]
        shape = [CI, len(taps), CO]
        tsl = slice(taps[0] % 3, taps[0] % 3 + len(taps))
        wsl = slice(3 * g + taps[0] % 3, 3 * g + taps[0] % 3 + len(taps))
        t0 = cmbp.tile(shape, F16, tag="cmb_t")
        nc.vector.tensor_scalar(
            out=t0, in0=wg_sb[g][:, 0, tsl, :], scalar1=a[:, 0:1],
            scalar2=None, op0=mybir.AluOpType.mult)
        t1 = cmbp.tile(shape, F16, tag="cmb_t")
        nc.vector.scalar_tensor_tensor(
            out=t1, in0=wg_sb[g][:, 1, tsl, :], scalar=a[:, 1:2], in1=t0,
            op0=mybir.AluOpType.mult, op1=mybir.AluOpType.add)
        t2 = cmbp.tile(shape, F16, tag="cmb_t")
        nc.vector.scalar_tensor_tensor(
            out=t2, in0=wg_sb[g][:, 2, tsl, :], scalar=a[:, 2:3], in1=t1,
            op0=mybir.AluOpType.mult, op1=mybir.AluOpType.add)
        nc.vector.scalar_tensor_tensor(
            out=weff[b][:, wsl, :], in0=wg_sb[g][:, 3, tsl, :],
            scalar=a[:, 3:4], in1=t2,
            op0=mybir.AluOpType.mult, op1=mybir.AluOpType.add)

    weff = [weffp.tile([CI, 9, CO], F16, tag=f"weff{b}", name=f"weff{b}")
            for b in range(B)]

    def win(b, tap, h0):
        """rhs window [128, 8, 64] for tap=(ky,kx) at output rows h0..h0+8."""
        ky, kx = tap // 3, tap % 3
        base = (h0 + ky) * PITCH + kx
        v = xr[b][:, base:base + NCOL].rearrange("p (r c) -> p r c", c=PITCH)
        return v[:, :, 0:W]

    ev_half = {}

    def evict(b, j, ps, single):
        """Bias-add+fp16 into half an ev tile; image-0 blocks go out in
        pairs (one HWDGE descriptor-gen per 16 rows), image-1 blocks singly
        as each bank finishes so the DMA chains spread across the conv."""
        if single or j % 2 == 0:
            ev = evp.tile([CO, 512 if single else 1024], F16, tag="ev",
                          name=f"ev{b}_{j}")
            ev_half[(b, j)] = ev
        else:
            ev = ev_half[(b, j - 1)]
        half = ev[:, 0:512] if (single or j % 2 == 0) else ev[:, 512:1024]
        if j % 2 == 0:
            nc.scalar.activation(out=half, in_=ps[:, 0:512],
                                 func=mybir.ActivationFunctionType.Identity,
                                 bias=cb_all[:, b:b + 1],
                                 scale=r_all[:, b:b + 1])
        else:
            nc.vector.tensor_scalar(out=half, in0=ps[:, 0:512],
                                    scalar1=r_all[:, b:b + 1],
                                    scalar2=cb_all[:, b:b + 1],
                                    op0=mybir.AluOpType.mult,
                                    op1=mybir.AluOpType.add)
        if single or j % 2 == 1:
            h0 = j * BROWS if single else (j - 1) * BROWS
            nr = BROWS if single else 2 * BROWS
            dma_eng = nc.sync if (j // 2) % 2 == 0 else nc.scalar
            dma_eng.dma_start(out=y_d[b, :, h0:h0 + nr, :],
                              in_=ev.rearrange("p (r c) -> p r c", c=W))

    def conv_A(b, mid=None):
        """Image 0: tap-major over the 7 cv banks (pipelines with the
        combine groups); mid() emitted after tap 6."""
        pss = [cvp.tile([128, 512], F32, tag="cv", name=f"cv{b}_{j}")
               for j in range(7)]
        for t in range(9):
            lhsT = weff[b][:, t, :]
            for j, ps in enumerate(pss):
                nc.tensor.matmul(ps[:, 0:512], lhsT, win(b, t, j * BROWS),
                                 start=(t == 0), stop=(t == 8))
                if t == 8:
                    evict(b, j, ps, False)
            if t == 6 and mid is not None:
                mid()

    def conv_A_bankmajor(b):
        """Image 1: bank-major — each bank's 9 taps run consecutively, so
        its evict+DMA streams out mid-conv instead of piling into the tail."""
        for j in range(7):
            ps = cvp.tile([128, 512], F32, tag="cv", name=f"cv{b}_{j}")
            for t in range(9):
                nc.tensor.matmul(ps[:, 0:512], weff[b][:, t, :],
                                 win(b, t, j * BROWS), start=(t == 0),
                                 stop=(t == 8))
            evict(b, j, ps, True)

    def winr(b, tap, h0, nr):
        ky, kx = tap // 3, tap % 3
        base = (h0 + ky) * PITCH + kx
        v = xr[b][:, base:base + nr * PITCH].rearrange("p (r c) -> p r c",
                                                       c=PITCH)
        return v[:, :, 0:W]

    def conv_B(b, last_img=False):
        """Last block (rows 56-63) on the shared tp bank. For the last
        image it is split 6+2, the tiny 2-row coda on a recycled cv bank,
        so the final evict+DMA chain after the last matmul is minimal."""
        if not last_img:
            ps = tpp.tile([128, 512], F32, tag="tp", name=f"cvB{b}")
            for t in range(9):
                nc.tensor.matmul(ps[:, 0:512], weff[b][:, t, :],
                                 win(b, t, 7 * BROWS), start=(t == 0),
                                 stop=(t == 8))
            evict(b, 7, ps, last_img)
            return
        psa = tpp.tile([128, 512], F32, tag="tp", name=f"cvBa{b}")
        for t in range(9):
            nc.tensor.matmul(psa[:, 0:384], weff[b][:, t, :],
                             winr(b, t, 56, 6), start=(t == 0), stop=(t == 8))
        eva = evp.tile([CO, 384], F16, tag="ev", name=f"evBa{b}")
        nc.scalar.activation(out=eva, in_=psa[:, 0:384],
                             func=mybir.ActivationFunctionType.Identity,
                             bias=cb_all[:, b:b + 1], scale=r_all[:, b:b + 1])
        nc.scalar.dma_start(out=y_d[b, :, 56:62, :],
                            in_=eva.rearrange("p (r c) -> p r c", c=W))
        psb = cvp.tile([128, 512], F32, tag="cv", name=f"cvBb{b}")
        for t in range(9):
            nc.tensor.matmul(psb[:, 0:128], weff[b][:, t, :],
                             winr(b, t, 62, 2), start=(t == 0), stop=(t == 8))
        evb = evp.tile([CO, 128], F16, tag="ev", name=f"evBb{b}")
        nc.vector.tensor_scalar(out=evb, in0=psb[:, 0:128],
                                scalar1=r_all[:, b:b + 1],
                                scalar2=cb_all[:, b:b + 1],
                                op0=mybir.AluOpType.mult,
                                op1=mybir.AluOpType.add)
        nc.sync.dma_start(out=y_d[b, :, 62:64, :],
                          in_=evb.rearrange("p (r c) -> p r c", c=W))

    # ---- program ----
    dummies(11)            # p-state ramp until pooled(0) is ready (~6.3us)
    reduce_image(0)
    reduce_image(1)        # on GPSIMD, pooled(1) ready mid-conv0
    se_attn(0)
    combine(0, 0, [0])     # per-tap for group 0: tap 0 ready ~0.8us sooner
    combine(0, 0, [1])
    combine(0, 0, [2])
    combine(0, 1, [3, 4, 5])
    combine(0, 2, [6, 7, 8])
    emit_cb(0)

    def image1_prep():
        se_attn(1)
        for g in range(3):
            combine(1, g, [3 * g, 3 * g + 1, 3 * g + 2])
        emit_cb(1)

    conv_A(0, mid=image1_prep)
    conv_B(0)
    conv_A_bankmajor(1)
    conv_B(1, last_img=True)


def get_nc():
    if "nc" not in _NC_CACHE:
        _NC_CACHE["nc"] = build_nc()
    return _NC_CACHE["nc"]


def shard_inputs(x, weight, bias, se_w1, se_w2, se_b2):
    x = np.asarray(x, np.float32)
    # host-side zero-pad into the flat pitch-65 fp16 layout
    xp = np.zeros((B_TOTAL, CI, 66, PITCH), np.float16)
    xp[:, :, 1:65, 1:65] = x
    xp = np.concatenate(
        [xp.reshape(B_TOTAL, CI, 66 * PITCH),
         np.zeros((B_TOTAL, CI, XPL - 66 * PITCH), np.float16)], axis=2)
    # weights -> [ky][ci, k, kx, co] fp16 (lhsT layout, grouped by ky)
    w4 = np.asarray(weight, np.float32).reshape(K, CO, CI, 3, 3)
    wt = w4.transpose(2, 0, 3, 4, 1).astype(np.float16)  # [ci, k, ky, kx, co]
    common = {f"wg{g}": np.ascontiguousarray(wt[:, :, g]) for g in range(3)}
    blob = np.zeros((CI, BLOB_W), np.float32)
    blob[:, BLOB_W1T:BLOB_W1T + HID] = np.asarray(se_w1, np.float32).T
    blob[0:HID, BLOB_W2T:BLOB_W2T + K] = np.asarray(se_w2, np.float32).T
    blob[HID, BLOB_W2T:BLOB_W2T + K] = np.asarray(se_b2, np.float32)
    blob[:, BLOB_BCOS:BLOB_BCOS + K] = np.asarray(bias, np.float32).reshape(
        K, CO).T
    common["cblob"] = blob
    return [
        dict(xp=np.ascontiguousarray(xp[c * B:(c + 1) * B]), **common)
        for c in range(N_CORES)
    ]


def kernel(x, weight, bias, se_w1, se_w2, se_b2):
    nc = get_nc()
    in_maps = shard_inputs(x, weight, bias, se_w1, se_w2, se_b2)
    res = run_bass_kernel_spmd(nc, in_maps, core_ids=list(range(N_CORES)))
    return np.concatenate(
        [r["y2"].astype(np.float32) for r in res.results], axis=0)


# revision 54
# speedup vs baseline: 1.7334x; 1.0232x over previous
"""DyConv2d (dynamic convolution with SE attention) on 8 TRN2 NeuronCores.

Reference computation (per image):
    attn = softmax(MLP(global_avg_pool(x)) / T)            # [K=4]
    y    = conv3x3(x, W) + bias                            # W: [K*128, 128, 3, 3]
    out  = sum_k attn[k] * y[k]                            # [128, 64, 64]

Conv is linear in the weights, so out = conv3x3(x, sum_k attn[k] W_k) +
sum_k attn[k] bias_k: one 128->128 conv per image instead of 128->512 (4x
fewer FLOPs). Data-parallel over batch, 2 images per core.

Layout/precision strategy (tolerance is 2e-2; fp16 keeps us ~1e-3):
  * x is zero-padded HOST-side into the flat pitch-65 layout
    [66 rows x 65 cols + 4] as fp16: every DMA lands conv-ready (each row's
    right pad aliases the next row's left pad), no on-chip re-layout, and
    fp16 halves the DMA bytes. fp16 matmul runs at 1 column/cycle, same as
    fp32r, with ~1e-4 input rounding error.
  * weights are host-prepped to the lhsT layout [ky-group][k, ci, kx, co]
    fp16; the per-image combine (4 fused DVE ops per ky-group) emits weff
    fp16 so the conv can start after group 0.
  * output is written fp16 and upcast to f32 on the host (~5e-4 of scale).

Schedule (single core, per-engine):
  * PE: warm-up matmuls on zeros from t~0.5us keep the PE busy so the
    p-state ramp (0.65 -> 2.4 GHz over 3us) completes before the first conv
    matmul; the SE matmuls slot between warm-up batches. Conv: 9 taps x 8
    row-blocks of [8 rows x 64 cols] = 512-column PSUM banks, tap-major so
    each bank accumulates all 9 taps; 7 "A" banks + 1 shared "B" bank
    (which also serves the tiny SE matmuls between images).
  * SE without DRAM bounces: logits come out as a [1,K] row by swapping
    matmul operands (lhsT=h, rhs=w2t); softmax uses host-precomputed
    exp(b2/T); attn is broadcast to 128 partitions with a ones-lhsT matmul.
  * DVE: per-half pooled reduction as x lands, weight combine, half the
    PSUM evictions. ACT: relu/exp, the other half of evictions (fused
    identity+bias+fp16-convert).
  * DMA: inputs on the sync queue (x img0 halves, weight group 0, x img1,
    groups 1-2), consts as one packed blob on the scalar queue; per-block
    output DMAs alternate sync/scalar.
"""

import sys

sys.path.insert(0, "/opt/trn_rl_repo")

import numpy as np

from concourse import bacc, mybir
import concourse.tile as tile
from concourse.bass_utils import run_bass_kernel_spmd

B_TOTAL = 16
N_CORES = 8
B = B_TOTAL // N_CORES  # images per core
CI = 128
CO = 128
K = 4
H = W = 64
HID = 33
TEMP = 30.0
F32 = mybir.dt.float32
F16 = mybir.dt.float16

PITCH = 65
XPL = PITCH * 66 + 4  # padded-x flat length (extra zeros absorb overrun)
NBLK = 8              # row blocks of 8 rows -> N=512 = one PSUM bank
BROWS = 8
NCOL = BROWS * PITCH  # 520 flat elements spanned by one block window

# const blob layout (f32, [128, BLOB_W]): w1t | w2aug | bias_cos
BLOB_W1T = 0                 # [128, 33]
BLOB_W2T = 33                # [34, 4] in partitions 0:34 (row 33 = se_b2)
BLOB_BCOS = 37               # [128, 4]
BLOB_W = 41

_NC_CACHE = {}


def build_nc():
    nc = bacc.Bacc("TRN2", target_bir_lowering=False)

    x_d = nc.dram_tensor("xp", [B, CI, XPL], F16, kind="ExternalInput")
    # weights grouped by ky: [ky][ci, k, kx, co] fp16 (partition-major,
    # matching the SBUF tile layout)
    wg_d = [nc.dram_tensor(f"wg{g}", [CI, K, 3, CO], F16, kind="ExternalInput")
            for g in range(3)]
    blob_d = nc.dram_tensor("cblob", [CI, BLOB_W], F32, kind="ExternalInput")
    y_d = nc.dram_tensor("y2", [B, CO, H, W], F16, kind="ExternalOutput")

    with tile.TileContext(nc) as tc:
        with (
            tc.tile_pool(name="consts", bufs=1) as consts,
            tc.tile_pool(name="ximg", bufs=2) as ximg,
            tc.tile_pool(name="weff", bufs=2) as weffp,
            tc.tile_pool(name="cmb", bufs=2) as cmbp,
            tc.tile_pool(name="sesb", bufs=2) as sesb,
            tc.tile_pool(name="ev", bufs=6) as evp,
            tc.tile_pool(name="cv", bufs=7, space="PSUM") as cvp,
            tc.tile_pool(name="tp", bufs=1, space="PSUM") as tpp,
        ):
            build_body(nc, tc, consts, ximg, weffp, cmbp, sesb, evp, cvp,
                       tpp, x_d, wg_d, blob_d, y_d)

    nc.compile()
    return nc


def build_body(nc, tc, consts, ximg, weffp, cmbp, sesb, evp, cvp, tpp,
               x_d, wg_d, blob_d, y_d):
    # ---- input DMAs (sync queue; x image 0 first, then wg0, x1, wg1-2) ----
    xr = [ximg.tile([CI, XPL], F16, tag=f"xr{b}", name=f"xr{b}")
          for b in range(B)]
    # image 0 in 3 pieces (small last piece => pooled available sooner);
    # chunked pooled reductions below are aligned to these piece bounds
    X0CUTS = [0, 1074, 2147, 3500, XPL]   # reduce-chunk bounds
    X1CUTS = [0, 1074, 2147, 3221, XPL]
    for lo, hi in [(0, 2147), (2147, 3500), (3500, XPL)]:
        nc.sync.dma_start(out=xr[0][:, lo:hi], in_=x_d[0, :, lo:hi])
    wg_sb = [consts.tile([CI, K, 3, CO], F16, tag=f"wg{g}", name=f"wg{g}")
             for g in range(3)]
    nc.sync.dma_start(out=wg_sb[0], in_=wg_d[0][:, :, :, :])
    nc.sync.dma_start(out=wg_sb[1], in_=wg_d[1][:, :, :, :])
    nc.sync.dma_start(out=wg_sb[2], in_=wg_d[2][:, :, :, :])
    # x image 1 last: its reduce chunks become DVE-ready only after image
    # 0's critical combine chains are done, so they can't delay the conv
    for lo, hi in [(0, 2147), (2147, XPL)]:
        nc.sync.dma_start(out=xr[1][:, lo:hi], in_=x_d[1, :, lo:hi])

    blob = consts.tile([CI, BLOB_W], F32, tag="blob")
    nc.scalar.dma_start(out=blob, in_=blob_d[:, :])
    w1t_sb = blob[:, BLOB_W1T:BLOB_W1T + HID]
    w2aug_sb = blob[0:HID + 1, BLOB_W2T:BLOB_W2T + K]
    bcos_sb = blob[:, BLOB_BCOS:BLOB_BCOS + K]

    # ---- PE warm-up: zero matmuls keep the p-state ramp going ----
    zl = consts.tile([CI, CO], F16, tag="zlhs")
    nc.gpsimd.memset(zl, 0.0)
    zr = consts.tile([CI, 512], F16, tag="zrhs")
    nc.gpsimd.memset(zr, 0.0)

    def dummies(n, pool=None):
        # mid-chain warm-up batches allocate from the cv pool: the tp bank
        # is held by SE PSUM tiles whose readers would stall a fresh alloc
        for _ in range(n):
            ps = (pool or tpp).tile([128, 512], F32, tag="cv" if pool
                                    else "tp", name="warm")
            nc.tensor.matmul(ps, zl, zr, start=True, stop=True)

    # ---- per-image pooled sums: chunked copy+accum on DVE (immediate
    # scalars get the fast DVE modes; small chunks can't block the
    # scheduler's critical ops) ----
    pooled = consts.tile([CI, B], F32, tag="pooled")
    NRED = 4
    pparts = consts.tile([CI, B, NRED], F32, tag="pparts")
    rscr = [consts.tile([CI, 1400], F16, tag=f"rscr{b}", name=f"rscr{b}")
            for b in range(B)]

    def reduce_image(b):
        cuts = X0CUTS if b == 0 else X1CUTS
        if b == 1:
            # forced WAW guard: image-1 chunks may only start after image
            # 0's last combine-group write, keeping them out of the
            # earliest-ready DVE scheduler's critical window
            nc.vector.tensor_scalar(
                out=rscr[1][:, 0:1], in0=weff[0][:, 8, 0:1], scalar1=0.0,
                scalar2=None, op0=mybir.AluOpType.mult)
        for i in range(NRED):
            o, n = cuts[i], cuts[i + 1] - cuts[i]
            nc.vector.tensor_scalar(
                out=rscr[b][:, 0:n], in0=xr[b][:, o:o + n],
                scalar1=1.0, scalar2=0.0,
                op0=mybir.AluOpType.mult, op1=mybir.AluOpType.add,
                accum_out=pparts[:, b, i:i + 1])
        nc.vector.reduce_sum(out=pooled[:, b:b + 1], in_=pparts[:, b, :],
                             axis=mybir.AxisListType.X)

    e_all = consts.tile([CI, K, B], F32, tag="e_all")
    r_all = consts.tile([CI, B], F32, tag="r_all")
    cb_all = consts.tile([CI, B], F32, tag="cb_all")

    def se_attn(b):
        """SE MLP -> raw softmax exponentials e_all[:, :, b] (broadcast on
        all partitions) and r_all[:, b] = 1/sum(e).

        h is replicated across 128 columns with a stride-0 read in the relu,
        so mm_lg (lhsT = [h; 1] augmented with a ones row that folds se_b2
        into the logits) directly yields logits broadcast over partitions.
        The weight combine consumes e directly (no normalize on the critical
        path); the 1/sum(e) factor is applied by each eviction's scale.
        """
        ps_h = tpp.tile([128, 512], F32, tag="tp", name=f"ps_h{b}")[0:HID, 0:1]
        nc.tensor.matmul(ps_h, w1t_sb, pooled[:, b:b + 1], start=True,
                         stop=True)
        h_aug = sesb.tile([HID + 1, CO], F32, tag="h_aug")
        nc.vector.memset(h_aug, 1.0)  # row HID stays 1.0 (folds se_b2 in)
        nc.scalar.activation(out=h_aug[0:HID, :],
                             in_=ps_h.broadcast_to([HID, CO]),
                             func=mybir.ActivationFunctionType.Relu,
                             scale=1.0 / (H * W))
        ps_lg = tpp.tile([128, 512], F32, tag="tp", name=f"ps_lg{b}")[:, 0:K]
        nc.tensor.matmul(ps_lg, h_aug, w2aug_sb, start=True, stop=True)
        nc.scalar.activation(out=e_all[:, :, b], in_=ps_lg,
                             func=mybir.ActivationFunctionType.Exp,
                             scale=1.0 / TEMP)
        s_sb = sesb.tile([CI, 1], F32, tag="s_sb")
        nc.vector.reduce_sum(out=s_sb, in_=e_all[:, :, b],
                             axis=mybir.AxisListType.X)
        nc.vector.reciprocal(out=r_all[:, b:b + 1], in_=s_sb)

    def emit_cb(b):
        # combined bias cb = r * sum_k e[k]*bias[k*CO+co] (emitted after the
        # critical combine chains; needed only at eviction time)
        tmp = sesb.tile([CI, K], F32, tag="cbtmp", name=f"cbt{b}")
        nc.vector.tensor_mul(tmp, bcos_sb, e_all[:, :, b])
        nc.vector.tensor_reduce(out=cb_all[:, b:b + 1], in_=tmp,
                                axis=mybir.AxisListType.X,
                                op=mybir.AluOpType.add)
        nc.vector.tensor_scalar_mul(cb_all[:, b:b + 1], cb_all[:, b:b + 1],
                                    r_all[:, b:b + 1])

    def combine(b, g, taps):
        """weff[b][:, taps, :] = sum_k e[k] * wg_sb[g][:, k, taps%3, :]"""
        a = e_all[:, :, b]
        shape = [CI, len(taps), CO]
        tsl = slice(taps[0] % 3, taps[0] % 3 + len(taps))
        wsl = slice(3 * g + taps[0] % 3, 3 * g + taps[0] % 3 + len(taps))
        t0 = cmbp.tile(shape, F16, tag="cmb_t")
        nc.vector.tensor_scalar(
            out=t0, in0=wg_sb[g][:, 0, tsl, :], scalar1=a[:, 0:1],
            scalar2=None, op0=mybir.AluOpType.mult)
        t1 = cmbp.tile(shape, F16, tag="cmb_t")
        nc.vector.scalar_tensor_tensor(
            out=t1, in0=wg_sb[g][:, 1, tsl, :], scalar=a[:, 1:2], in1=t0,
            op0=mybir.AluOpType.mult, op1=mybir.AluOpType.add)
        t2 = cmbp.tile(shape, F16, tag="cmb_t")
        nc.vector.scalar_tensor_tensor(
            out=t2, in0=wg_sb[g][:, 2, tsl, :], scalar=a[:, 2:3], in1=t1,
            op0=mybir.AluOpType.mult, op1=mybir.AluOpType.add)
        nc.vector.scalar_tensor_tensor(
            out=weff[b][:, wsl, :], in0=wg_sb[g][:, 3, tsl, :],
            scalar=a[:, 3:4], in1=t2,
            op0=mybir.AluOpType.mult, op1=mybir.AluOpType.add)

    weff = [weffp.tile([CI, 9, CO], F16, tag=f"weff{b}", name=f"weff{b}")
            for b in range(B)]

    def win(b, tap, h0):
        """rhs window [128, 8, 64] for tap=(ky,kx) at output rows h0..h0+8."""
        ky, kx = tap // 3, tap % 3
        base = (h0 + ky) * PITCH + kx
        v = xr[b][:, base:base + NCOL].rearrange("p (r c) -> p r c", c=PITCH)
        return v[:, :, 0:W]

    ev_half = {}

    def evict(b, j, ps, single):
        """Bias-add+fp16 into half an ev tile; image-0 blocks go out in
        pairs (one HWDGE descriptor-gen per 16 rows), image-1 blocks singly
        as each bank finishes so the DMA chains spread across the conv."""
        if single or j % 2 == 0:
            ev = evp.tile([CO, 512 if single else 1024], F16, tag="ev",
                          name=f"ev{b}_{j}")
            ev_half[(b, j)] = ev
        else:
            ev = ev_half[(b, j - 1)]
        half = ev[:, 0:512] if (single or j % 2 == 0) else ev[:, 512:1024]
        if j % 2 == 0:
            nc.scalar.activation(out=half, in_=ps[:, 0:512],
                                 func=mybir.ActivationFunctionType.Identity,
                                 bias=cb_all[:, b:b + 1],
                                 scale=r_all[:, b:b + 1])
        else:
            nc.vector.tensor_scalar(out=half, in0=ps[:, 0:512],
                                    scalar1=r_all[:, b:b + 1],
                                    scalar2=cb_all[:, b:b + 1],
                                    op0=mybir.AluOpType.mult,
                                    op1=mybir.AluOpType.add)
        if single or j % 2 == 1:
            h0 = j * BROWS if single else (j - 1) * BROWS
            nr = BROWS if single else 2 * BROWS
            dma_eng = nc.sync if (j // 2) % 2 == 0 else nc.scalar
            dma_eng.dma_start(out=y_d[b, :, h0:h0 + nr, :],
                              in_=ev.rearrange("p (r c) -> p r c", c=W))

    def conv_A(b, mid=None):
        """Image 0: tap-major over the 7 cv banks (pipelines with the
        combine groups); mid() emitted after tap 6."""
        pss = [cvp.tile([128, 512], F32, tag="cv", name=f"cv{b}_{j}")
               for j in range(7)]
        for t in range(9):
            lhsT = weff[b][:, t, :]
            for j, ps in enumerate(pss):
                nc.tensor.matmul(ps[:, 0:512], lhsT, win(b, t, j * BROWS),
                                 start=(t == 0), stop=(t == 8))
                if t == 8:
                    evict(b, j, ps, False)
            if t == 6 and mid is not None:
                mid()

    def conv_A_bankmajor(b):
        """Image 1: bank-major — each bank's 9 taps run consecutively, so
        its evict+DMA streams out mid-conv instead of piling into the tail."""
        for j in range(7):
            ps = cvp.tile([128, 512], F32, tag="cv", name=f"cv{b}_{j}")
            for t in range(9):
                nc.tensor.matmul(ps[:, 0:512], weff[b][:, t, :],
                                 win(b, t, j * BROWS), start=(t == 0),
                                 stop=(t == 8))
            evict(b, j, ps, True)

    def winr(b, tap, h0, nr):
        ky, kx = tap // 3, tap % 3
        base = (h0 + ky) * PITCH + kx
        v = xr[b][:, base:base + nr * PITCH].rearrange("p (r c) -> p r c",
                                                       c=PITCH)
        return v[:, :, 0:W]

    def conv_B(b, last_img=False):
        """Last block (rows 56-63) on the shared tp bank. For the last
        image it is split 6+2, the tiny 2-row coda on a recycled cv bank,
        so the final evict+DMA chain after the last matmul is minimal."""
        if not last_img:
            ps = tpp.tile([128, 512], F32, tag="tp", name=f"cvB{b}")
            for t in range(9):
                nc.tensor.matmul(ps[:, 0:512], weff[b][:, t, :],
                                 win(b, t, 7 * BROWS), start=(t == 0),
                                 stop=(t == 8))
            evict(b, 7, ps, last_img)
            return
        psa = tpp.tile([128, 512], F32, tag="tp", name=f"cvBa{b}")
        for t in range(9):
            nc.tensor.matmul(psa[:, 0:448], weff[b][:, t, :],
                             winr(b, t, 56, 7), start=(t == 0), stop=(t == 8))
        ev = evp.tile([CO, 512], F16, tag="ev", name=f"evB{b}")
        nc.scalar.activation(out=ev[:, 0:448], in_=psa[:, 0:448],
                             func=mybir.ActivationFunctionType.Identity,
                             bias=cb_all[:, b:b + 1], scale=r_all[:, b:b + 1])
        psb = cvp.tile([128, 512], F32, tag="cv", name=f"cvBb{b}")
        for t in range(9):
            nc.tensor.matmul(psb[:, 0:64], weff[b][:, t, :],
                             winr(b, t, 63, 1), start=(t == 0), stop=(t == 8))
        nc.vector.tensor_scalar(out=ev[:, 448:512], in0=psb[:, 0:64],
                                scalar1=r_all[:, b:b + 1],
                                scalar2=cb_all[:, b:b + 1],
                                op0=mybir.AluOpType.mult,
                                op1=mybir.AluOpType.add)
        nc.sync.dma_start(out=y_d[b, :, 56:64, :],
                          in_=ev.rearrange("p (r c) -> p r c", c=W))

    # ---- program ----
    dummies(11)            # p-state ramp until pooled(0) is ready (~6.3us)
    reduce_image(0)
    se_attn(0)
    combine(0, 0, [0])     # per-tap for group 0: tap 0 ready ~0.8us sooner
    combine(0, 0, [1])
    combine(0, 0, [2])
    combine(0, 1, [3, 4, 5])
    combine(0, 2, [6, 7, 8])
    reduce_image(1)        # chunks guarded behind image-0's last combine
    emit_cb(0)

    def image1_prep():
        se_attn(1)
        for g in range(3):
            combine(1, g, [3 * g, 3 * g + 1, 3 * g + 2])
        emit_cb(1)

    conv_A(0, mid=image1_prep)
    conv_B(0)
    conv_A_bankmajor(1)
    conv_B(1, last_img=True)


def get_nc():
    if "nc" not in _NC_CACHE:
        _NC_CACHE["nc"] = build_nc()
    return _NC_CACHE["nc"]


def shard_inputs(x, weight, bias, se_w1, se_w2, se_b2):
    x = np.asarray(x, np.float32)
    # host-side zero-pad into the flat pitch-65 fp16 layout
    xp = np.zeros((B_TOTAL, CI, 66, PITCH), np.float16)
    xp[:, :, 1:65, 1:65] = x
    xp = np.concatenate(
        [xp.reshape(B_TOTAL, CI, 66 * PITCH),
         np.zeros((B_TOTAL, CI, XPL - 66 * PITCH), np.float16)], axis=2)
    # weights -> [ky][ci, k, kx, co] fp16 (lhsT layout, grouped by ky)
    w4 = np.asarray(weight, np.float32).reshape(K, CO, CI, 3, 3)
    wt = w4.transpose(2, 0, 3, 4, 1).astype(np.float16)  # [ci, k, ky, kx, co]
    common = {f"wg{g}": np.ascontiguousarray(wt[:, :, g]) for g in range(3)}
    blob = np.zeros((CI, BLOB_W), np.float32)
    blob[:, BLOB_W1T:BLOB_W1T + HID] = np.asarray(se_w1, np.float32).T
    blob[0:HID, BLOB_W2T:BLOB_W2T + K] = np.asarray(se_w2, np.float32).T
    blob[HID, BLOB_W2T:BLOB_W2T + K] = np.asarray(se_b2, np.float32)
    blob[:, BLOB_BCOS:BLOB_BCOS + K] = np.asarray(bias, np.float32).reshape(
        K, CO).T
    common["cblob"] = blob
    return [
        dict(xp=np.ascontiguousarray(xp[c * B:(c + 1) * B]), **common)
        for c in range(N_CORES)
    ]


def kernel(x, weight, bias, se_w1, se_w2, se_b2):
    nc = get_nc()
    in_maps = shard_inputs(x, weight, bias, se_w1, se_w2, se_b2)
    res = run_bass_kernel_spmd(nc, in_maps, core_ids=list(range(N_CORES)))
    return np.concatenate(
        [r["y2"].astype(np.float32) for r in res.results], axis=0)


# revision 65
# speedup vs baseline: 1.7369x; 1.0021x over previous
"""DyConv2d (dynamic convolution with SE attention) on 8 TRN2 NeuronCores.

Reference computation (per image):
    attn = softmax(MLP(global_avg_pool(x)) / T)            # [K=4]
    y    = conv3x3(x, W) + bias                            # W: [K*128, 128, 3, 3]
    out  = sum_k attn[k] * y[k]                            # [128, 64, 64]

Conv is linear in the weights, so out = conv3x3(x, sum_k attn[k] W_k) +
sum_k attn[k] bias_k: one 128->128 conv per image instead of 128->512 (4x
fewer FLOPs). Data-parallel over batch, 2 images per core.

Layout/precision strategy (tolerance is 2e-2; fp16 keeps us ~1e-3):
  * x is zero-padded HOST-side into the flat pitch-65 layout
    [66 rows x 65 cols + 4] as fp16: every DMA lands conv-ready (each row's
    right pad aliases the next row's left pad), no on-chip re-layout, and
    fp16 halves the DMA bytes. fp16 matmul runs at 1 column/cycle, same as
    fp32r, with ~1e-4 input rounding error.
  * weights are host-prepped to the lhsT layout [ky-group][k, ci, kx, co]
    fp16; the per-image combine (4 fused DVE ops per ky-group) emits weff
    fp16 so the conv can start after group 0.
  * output is written fp16 and upcast to f32 on the host (~5e-4 of scale).

Schedule (single core, per-engine):
  * PE: warm-up matmuls on zeros from t~0.5us keep the PE busy so the
    p-state ramp (0.65 -> 2.4 GHz over 3us) completes before the first conv
    matmul; the SE matmuls slot between warm-up batches. Conv: 9 taps x 8
    row-blocks of [8 rows x 64 cols] = 512-column PSUM banks, tap-major so
    each bank accumulates all 9 taps; 7 "A" banks + 1 shared "B" bank
    (which also serves the tiny SE matmuls between images).
  * SE without DRAM bounces: logits come out as a [1,K] row by swapping
    matmul operands (lhsT=h, rhs=w2t); softmax uses host-precomputed
    exp(b2/T); attn is broadcast to 128 partitions with a ones-lhsT matmul.
  * DVE: per-half pooled reduction as x lands, weight combine, half the
    PSUM evictions. ACT: relu/exp, the other half of evictions (fused
    identity+bias+fp16-convert).
  * DMA: inputs on the sync queue (x img0 halves, weight group 0, x img1,
    groups 1-2), consts as one packed blob on the scalar queue; per-block
    output DMAs alternate sync/scalar.
"""

import sys

sys.path.insert(0, "/opt/trn_rl_repo")

import numpy as np

from concourse import bacc, mybir
import concourse.tile as tile
from concourse.bass_utils import run_bass_kernel_spmd

B_TOTAL = 16
N_CORES = 8
B = B_TOTAL // N_CORES  # images per core
CI = 128
CO = 128
K = 4
H = W = 64
HID = 33
TEMP = 30.0
F32 = mybir.dt.float32
F16 = mybir.dt.float16

PITCH = 65
XPL = PITCH * 66 + 4  # padded-x flat length (extra zeros absorb overrun)
NBLK = 8              # row blocks of 8 rows -> N=512 = one PSUM bank
BROWS = 8
NCOL = BROWS * PITCH  # 520 flat elements spanned by one block window

# const blob layout (f32, [128, BLOB_W]): w1t | w2aug | bias_cos
BLOB_W1T = 0                 # [128, 33]
BLOB_W2T = 33                # [34, 4] in partitions 0:34 (row 33 = se_b2)
BLOB_BCOS = 37               # [128, 4]
BLOB_W = 41

_NC_CACHE = {}


def build_nc():
    nc = bacc.Bacc("TRN2", target_bir_lowering=False)

    x_d = nc.dram_tensor("xp", [B, CI, XPL], F16, kind="ExternalInput")
    # weights grouped by ky: [ky][ci, k, kx, co] fp16 (partition-major,
    # matching the SBUF tile layout)
    wg_d = [nc.dram_tensor(f"wg{g}", [CI, K, 3, CO], F16, kind="ExternalInput")
            for g in range(3)]
    blob_d = nc.dram_tensor("cblob", [CI, BLOB_W], F32, kind="ExternalInput")
    y_d = nc.dram_tensor("y2", [B, CO, H, W], F16, kind="ExternalOutput")

    with tile.TileContext(nc) as tc:
        with (
            tc.tile_pool(name="consts", bufs=1) as consts,
            tc.tile_pool(name="ximg", bufs=2) as ximg,
            tc.tile_pool(name="weff", bufs=2) as weffp,
            tc.tile_pool(name="cmb", bufs=2) as cmbp,
            tc.tile_pool(name="sesb", bufs=2) as sesb,
            tc.tile_pool(name="ev", bufs=6) as evp,
            tc.tile_pool(name="cv", bufs=7, space="PSUM") as cvp,
            tc.tile_pool(name="tp", bufs=1, space="PSUM") as tpp,
        ):
            build_body(nc, tc, consts, ximg, weffp, cmbp, sesb, evp, cvp,
                       tpp, x_d, wg_d, blob_d, y_d)

    nc.compile()
    return nc


def build_body(nc, tc, consts, ximg, weffp, cmbp, sesb, evp, cvp, tpp,
               x_d, wg_d, blob_d, y_d):
    # ---- input DMAs (sync queue; x image 0 first, then wg0, x1, wg1-2) ----
    xr = [ximg.tile([CI, XPL], F16, tag=f"xr{b}", name=f"xr{b}")
          for b in range(B)]
    # image 0 in 3 pieces (small last piece => pooled available sooner);
    # chunked pooled reductions below are aligned to these piece bounds
    X0CUTS = [0, 1074, 2147, 3500, XPL]   # reduce-chunk bounds
    X1CUTS = [0, 1074, 2147, 3221, XPL]
    for lo, hi in [(0, 2147), (2147, 3500), (3500, XPL)]:
        nc.sync.dma_start(out=xr[0][:, lo:hi], in_=x_d[0, :, lo:hi])
    wg_sb = [consts.tile([CI, K, 3, CO], F16, tag=f"wg{g}", name=f"wg{g}")
             for g in range(3)]
    nc.sync.dma_start(out=wg_sb[0], in_=wg_d[0][:, :, :, :])
    nc.sync.dma_start(out=wg_sb[1], in_=wg_d[1][:, :, :, :])
    nc.sync.dma_start(out=wg_sb[2], in_=wg_d[2][:, :, :, :])
    # x image 1 last: its reduce chunks become DVE-ready only after image
    # 0's critical combine chains are done, so they can't delay the conv
    for lo, hi in [(0, 2147), (2147, XPL)]:
        nc.sync.dma_start(out=xr[1][:, lo:hi], in_=x_d[1, :, lo:hi])

    blob = consts.tile([CI, BLOB_W], F32, tag="blob")
    nc.scalar.dma_start(out=blob, in_=blob_d[:, :])
    w1t_sb = blob[:, BLOB_W1T:BLOB_W1T + HID]
    w2aug_sb = blob[0:HID + 1, BLOB_W2T:BLOB_W2T + K]
    bcos_sb = blob[:, BLOB_BCOS:BLOB_BCOS + K]

    # ---- PE warm-up: zero matmuls keep the p-state ramp going ----
    zl = consts.tile([CI, CO], F16, tag="zlhs")
    nc.gpsimd.memset(zl, 0.0)
    zr = consts.tile([CI, 512], F16, tag="zrhs")
    nc.gpsimd.memset(zr, 0.0)

    def dummies(n, pool=None):
        # mid-chain warm-up batches allocate from the cv pool: the tp bank
        # is held by SE PSUM tiles whose readers would stall a fresh alloc
        for _ in range(n):
            ps = (pool or tpp).tile([128, 512], F32, tag="cv" if pool
                                    else "tp", name="warm")
            nc.tensor.matmul(ps, zl, zr, start=True, stop=True)

    # ---- per-image pooled sums: chunked copy+accum on DVE (immediate
    # scalars get the fast DVE modes; small chunks can't block the
    # scheduler's critical ops) ----
    pooled = consts.tile([CI, B], F32, tag="pooled")
    pparts = consts.tile([CI, B, 5], F32, tag="pparts")
    rscr = [consts.tile([CI, 1400], F16, tag=f"rscr{b}", name=f"rscr{b}")
            for b in range(B)]

    def reduce_image(b):
        cuts = X0CUTS if b == 0 else X1CUTS
        if b == 1:
            # forced WAW guard: image-1 chunks may only start after image
            # 0's last combine-group write, keeping them out of the
            # earliest-ready DVE scheduler's critical window
            nc.vector.tensor_scalar(
                out=rscr[1][:, 0:1], in0=weff[0][:, 8, 0:1], scalar1=0.0,
                scalar2=None, op0=mybir.AluOpType.mult)
        for i in range(len(cuts) - 1):
            o, n = cuts[i], cuts[i + 1] - cuts[i]
            nc.vector.tensor_scalar(
                out=rscr[b][:, 0:n], in0=xr[b][:, o:o + n],
                scalar1=1.0, scalar2=0.0,
                op0=mybir.AluOpType.mult, op1=mybir.AluOpType.add,
                accum_out=pparts[:, b, i:i + 1])
        nc.vector.reduce_sum(out=pooled[:, b:b + 1],
                             in_=pparts[:, b, 0:len(cuts) - 1],
                             axis=mybir.AxisListType.X)

    e_all = consts.tile([CI, K, B], F32, tag="e_all")
    r_all = consts.tile([CI, B], F32, tag="r_all")
    cb_all = consts.tile([CI, B], F32, tag="cb_all")

    def se_attn(b):
        """SE MLP -> raw softmax exponentials e_all[:, :, b] (broadcast on
        all partitions) and r_all[:, b] = 1/sum(e).

        h is replicated across 128 columns with a stride-0 read in the relu,
        so mm_lg (lhsT = [h; 1] augmented with a ones row that folds se_b2
        into the logits) directly yields logits broadcast over partitions.
        The weight combine consumes e directly (no normalize on the critical
        path); the 1/sum(e) factor is applied by each eviction's scale.
        """
        ps_h = tpp.tile([128, 512], F32, tag="tp", name=f"ps_h{b}")[0:HID, 0:1]
        nc.tensor.matmul(ps_h, w1t_sb, pooled[:, b:b + 1], start=True,
                         stop=True)
        h_aug = sesb.tile([HID + 1, CO], F32, tag="h_aug")
        nc.vector.memset(h_aug, 1.0)  # row HID stays 1.0 (folds se_b2 in)
        nc.scalar.activation(out=h_aug[0:HID, :],
                             in_=ps_h.broadcast_to([HID, CO]),
                             func=mybir.ActivationFunctionType.Relu,
                             scale=1.0 / (H * W))
        ps_lg = tpp.tile([128, 512], F32, tag="tp", name=f"ps_lg{b}")[:, 0:K]
        nc.tensor.matmul(ps_lg, h_aug, w2aug_sb, start=True, stop=True)
        nc.scalar.activation(out=e_all[:, :, b], in_=ps_lg,
                             func=mybir.ActivationFunctionType.Exp,
                             scale=1.0 / TEMP)

    def emit_r(b):
        # r = 1/sum(e): only needed at eviction time, emitted after the
        # critical combine chain so it can't precede it in the DVE queue
        s_sb = sesb.tile([CI, 1], F32, tag="s_sb", name=f"s_sb{b}")
        nc.vector.reduce_sum(out=s_sb, in_=e_all[:, :, b],
                             axis=mybir.AxisListType.X)
        nc.vector.reciprocal(out=r_all[:, b:b + 1], in_=s_sb)

    def emit_cb(b):
        # combined bias cb = r * sum_k e[k]*bias[k*CO+co] (emitted after the
        # critical combine chains; needed only at eviction time)
        tmp = sesb.tile([CI, K], F32, tag="cbtmp", name=f"cbt{b}")
        nc.vector.tensor_mul(tmp, bcos_sb, e_all[:, :, b])
        nc.vector.tensor_reduce(out=cb_all[:, b:b + 1], in_=tmp,
                                axis=mybir.AxisListType.X,
                                op=mybir.AluOpType.add)
        nc.vector.tensor_scalar_mul(cb_all[:, b:b + 1], cb_all[:, b:b + 1],
                                    r_all[:, b:b + 1])

    def combine(b, g, taps):
        """weff[b][:, taps, :] = sum_k e[k] * wg_sb[g][:, k, taps%3, :]"""
        a = e_all[:, :, b]
        shape = [CI, len(taps), CO]
        tsl = slice(taps[0] % 3, taps[0] % 3 + len(taps))
        wsl = slice(3 * g + taps[0] % 3, 3 * g + taps[0] % 3 + len(taps))
        t0 = cmbp.tile(shape, F16, tag="cmb_t")
        nc.vector.tensor_scalar(
            out=t0, in0=wg_sb[g][:, 0, tsl, :], scalar1=a[:, 0:1],
            scalar2=None, op0=mybir.AluOpType.mult)
        t1 = cmbp.tile(shape, F16, tag="cmb_t")
        nc.vector.scalar_tensor_tensor(
            out=t1, in0=wg_sb[g][:, 1, tsl, :], scalar=a[:, 1:2], in1=t0,
            op0=mybir.AluOpType.mult, op1=mybir.AluOpType.add)
        t2 = cmbp.tile(shape, F16, tag="cmb_t")
        nc.vector.scalar_tensor_tensor(
            out=t2, in0=wg_sb[g][:, 2, tsl, :], scalar=a[:, 2:3], in1=t1,
            op0=mybir.AluOpType.mult, op1=mybir.AluOpType.add)
        nc.vector.scalar_tensor_tensor(
            out=weff[b][:, wsl, :], in0=wg_sb[g][:, 3, tsl, :],
            scalar=a[:, 3:4], in1=t2,
            op0=mybir.AluOpType.mult, op1=mybir.AluOpType.add)

    weff = [weffp.tile([CI, 9, CO], F16, tag=f"weff{b}", name=f"weff{b}")
            for b in range(B)]

    def win(b, tap, h0):
        """rhs window [128, 8, 64] for tap=(ky,kx) at output rows h0..h0+8."""
        ky, kx = tap // 3, tap % 3
        base = (h0 + ky) * PITCH + kx
        v = xr[b][:, base:base + NCOL].rearrange("p (r c) -> p r c", c=PITCH)
        return v[:, :, 0:W]

    ev_half = {}

    def evict(b, j, ps, single):
        """Bias-add+fp16 into half an ev tile; image-0 blocks go out in
        pairs (one HWDGE descriptor-gen per 16 rows), image-1 blocks singly
        as each bank finishes so the DMA chains spread across the conv."""
        if single or j % 2 == 0:
            ev = evp.tile([CO, 512 if single else 1024], F16, tag="ev",
                          name=f"ev{b}_{j}")
            ev_half[(b, j)] = ev
        else:
            ev = ev_half[(b, j - 1)]
        half = ev[:, 0:512] if (single or j % 2 == 0) else ev[:, 512:1024]
        if j % 2 == 0:
            nc.scalar.activation(out=half, in_=ps[:, 0:512],
                                 func=mybir.ActivationFunctionType.Identity,
                                 bias=cb_all[:, b:b + 1],
                                 scale=r_all[:, b:b + 1])
        else:
            nc.vector.tensor_scalar(out=half, in0=ps[:, 0:512],
                                    scalar1=r_all[:, b:b + 1],
                                    scalar2=cb_all[:, b:b + 1],
                                    op0=mybir.AluOpType.mult,
                                    op1=mybir.AluOpType.add)
        if single or j % 2 == 1:
            h0 = j * BROWS if single else (j - 1) * BROWS
            nr = BROWS if single else 2 * BROWS
            dma_eng = nc.sync if (j // 2) % 2 == 0 else nc.scalar
            dma_eng.dma_start(out=y_d[b, :, h0:h0 + nr, :],
                              in_=ev.rearrange("p (r c) -> p r c", c=W))

    def conv_A(b, mid=None):
        """Image 0: tap-major over the 7 cv banks (pipelines with the
        combine groups); mid() emitted after tap 6."""
        pss = [cvp.tile([128, 512], F32, tag="cv", name=f"cv{b}_{j}")
               for j in range(7)]
        for t in range(9):
            lhsT = weff[b][:, t, :]
            for j, ps in enumerate(pss):
                nc.tensor.matmul(ps[:, 0:512], lhsT, win(b, t, j * BROWS),
                                 start=(t == 0), stop=(t == 8))
                if t == 8:
                    evict(b, j, ps, False)
            if t == 6 and mid is not None:
                mid()

    def conv_A_bankmajor(b):
        """Image 1: bank-major — each bank's 9 taps run consecutively, so
        its evict+DMA streams out mid-conv instead of piling into the tail."""
        for j in range(7):
            ps = cvp.tile([128, 512], F32, tag="cv", name=f"cv{b}_{j}")
            for t in range(9):
                nc.tensor.matmul(ps[:, 0:512], weff[b][:, t, :],
                                 win(b, t, j * BROWS), start=(t == 0),
                                 stop=(t == 8))
            evict(b, j, ps, True)

    def winr(b, tap, h0, nr):
        ky, kx = tap // 3, tap % 3
        base = (h0 + ky) * PITCH + kx
        v = xr[b][:, base:base + nr * PITCH].rearrange("p (r c) -> p r c",
                                                       c=PITCH)
        return v[:, :, 0:W]

    def conv_B(b, last_img=False):
        """Last block (rows 56-63) on the shared tp bank. For the last
        image it is split 6+2, the tiny 2-row coda on a recycled cv bank,
        so the final evict+DMA chain after the last matmul is minimal."""
        if not last_img:
            ps = tpp.tile([128, 512], F32, tag="tp", name=f"cvB{b}")
            for t in range(9):
                nc.tensor.matmul(ps[:, 0:512], weff[b][:, t, :],
                                 win(b, t, 7 * BROWS), start=(t == 0),
                                 stop=(t == 8))
            evict(b, 7, ps, last_img)
            return
        psa = tpp.tile([128, 512], F32, tag="tp", name=f"cvBa{b}")
        for t in range(9):
            nc.tensor.matmul(psa[:, 0:448], weff[b][:, t, :],
                             winr(b, t, 56, 7), start=(t == 0), stop=(t == 8))
        ev = evp.tile([CO, 512], F16, tag="ev", name=f"evB{b}")
        nc.scalar.activation(out=ev[:, 0:448], in_=psa[:, 0:448],
                             func=mybir.ActivationFunctionType.Identity,
                             bias=cb_all[:, b:b + 1], scale=r_all[:, b:b + 1])
        psb = cvp.tile([128, 512], F32, tag="cv", name=f"cvBb{b}")
        for t in range(9):
            nc.tensor.matmul(psb[:, 0:64], weff[b][:, t, :],
                             winr(b, t, 63, 1), start=(t == 0), stop=(t == 8))
        nc.vector.tensor_scalar(out=ev[:, 448:512], in0=psb[:, 0:64],
                                scalar1=r_all[:, b:b + 1],
                                scalar2=cb_all[:, b:b + 1],
                                op0=mybir.AluOpType.mult,
                                op1=mybir.AluOpType.add)
        nc.sync.dma_start(out=y_d[b, :, 56:64, :],
                          in_=ev.rearrange("p (r c) -> p r c", c=W))

    # ---- program ----
    dummies(11)            # p-state ramp until pooled(0) is ready (~6.3us)
    reduce_image(0)
    se_attn(0)
    combine(0, 0, [0])     # per-tap for group 0: tap 0 ready ~0.8us sooner
    combine(0, 0, [1])
    combine(0, 0, [2])
    combine(0, 1, [3, 4, 5])
    combine(0, 2, [6, 7, 8])
    emit_r(0)
    reduce_image(1)        # chunks guarded behind image-0's last combine
    emit_cb(0)

    def image1_prep():
        se_attn(1)
        for g in range(3):
            combine(1, g, [3 * g, 3 * g + 1, 3 * g + 2])
        emit_r(1)
        emit_cb(1)

    conv_A(0, mid=image1_prep)
    conv_B(0)
    conv_A_bankmajor(1)
    conv_B(1, last_img=True)


def get_nc():
    if "nc" not in _NC_CACHE:
        _NC_CACHE["nc"] = build_nc()
    return _NC_CACHE["nc"]


def shard_inputs(x, weight, bias, se_w1, se_w2, se_b2):
    x = np.asarray(x, np.float32)
    # host-side zero-pad into the flat pitch-65 fp16 layout
    xp = np.zeros((B_TOTAL, CI, 66, PITCH), np.float16)
    xp[:, :, 1:65, 1:65] = x
    xp = np.concatenate(
        [xp.reshape(B_TOTAL, CI, 66 * PITCH),
         np.zeros((B_TOTAL, CI, XPL - 66 * PITCH), np.float16)], axis=2)
    # weights -> [ky][ci, k, kx, co] fp16 (lhsT layout, grouped by ky)
    w4 = np.asarray(weight, np.float32).reshape(K, CO, CI, 3, 3)
    wt = w4.transpose(2, 0, 3, 4, 1).astype(np.float16)  # [ci, k, ky, kx, co]
    common = {f"wg{g}": np.ascontiguousarray(wt[:, :, g]) for g in range(3)}
    blob = np.zeros((CI, BLOB_W), np.float32)
    blob[:, BLOB_W1T:BLOB_W1T + HID] = np.asarray(se_w1, np.float32).T
    blob[0:HID, BLOB_W2T:BLOB_W2T + K] = np.asarray(se_w2, np.float32).T
    blob[HID, BLOB_W2T:BLOB_W2T + K] = np.asarray(se_b2, np.float32)
    blob[:, BLOB_BCOS:BLOB_BCOS + K] = np.asarray(bias, np.float32).reshape(
        K, CO).T
    common["cblob"] = blob
    return [
        dict(xp=np.ascontiguousarray(xp[c * B:(c + 1) * B]), **common)
        for c in range(N_CORES)
    ]


def kernel(x, weight, bias, se_w1, se_w2, se_b2):
    nc = get_nc()
    in_maps = shard_inputs(x, weight, bias, se_w1, se_w2, se_b2)
    res = run_bass_kernel_spmd(nc, in_maps, core_ids=list(range(N_CORES)))
    return np.concatenate(
        [r["y2"].astype(np.float32) for r in res.results], axis=0)
